# revision 7
# baseline (speedup 1.0000x reference)
"""Dense causal attention block (QKV proj + RoPE + causal attention + out proj)
distributed over 8 TRN2 NeuronCores.

Sharding: tensor-parallel over heads (2 heads/core). Each core computes
q/k/v for its heads (q/k d-major so QK^T needs no transposes, v
token-major), causal attention in logits-transposed [k, q] layout, then
one AllToAll per (batch, token-half) converts head-sharding to
token-sharding for the output projection. Host assembles the 32 token
chunks.

Softmax denominator: instead of a per-key-block ones-matmul on the PE
(which costs as many PE columns as the PV matmul itself), the exp'd
weight blocks are accumulated across key blocks on the vector engine
(bf16, <=16-deep partial sums so rounding is negligible) and a single
ones-matmul per (batch, q-chunk, head) reduces the 128 partitions in
f32 PSUM.

Because attention alone is scalar-exp-bound (~686ns/block vs ~526ns of
PE work), independent GEMM work is interleaved into the attention
stream to keep the PE saturated: batch-1 V-projection chunks fill
batch-0 attention, and output-projection chunks (whose AllToAll landed
during the intervening batch-1 QKV phase) fill batch-1 attention.

All matmuls run in bf16 (f32 PSUM accumulate); host pre-casts and
pre-tiles the weights so every DMA is partition-contiguous.
"""
import numpy as np
import ml_dtypes

import concourse.bass as bass
import concourse.mybir as mybir
from concourse import tile, bacc
from concourse.bass_utils import run_bass_kernel_spmd

BF16 = ml_dtypes.bfloat16
B, S, D = 2, 2048, 2048
H, HD = 16, 128
NCORES = 8
HL = H // NCORES            # heads per core = 2
HDL = HL * HD               # local head dims = 256
T = B * S                   # 4096 tokens (b-major)
TCB = S // NCORES           # 256 tokens per core per batch after AllToAll
SCALE = float(1.0 / np.sqrt(HD))
BF = mybir.dt.bfloat16
F32 = mybir.dt.float32
AF = mybir.ActivationFunctionType


def build():
    nc = bacc.Bacc(None)
    xT = nc.declare_dram_parameter("xT", [D, T], BF, isOutput=False)
    # weights host-pretiled: [128, kb * out_cols] with kb-major columns
    wq = nc.declare_dram_parameter("wq", [128, 16 * HDL], BF, isOutput=False)
    wk = nc.declare_dram_parameter("wk", [128, 16 * HDL], BF, isOutput=False)
    wv = nc.declare_dram_parameter("wv", [128, 16 * HDL], BF, isOutput=False)
    wo = nc.declare_dram_parameter("wo", [128, 16 * D], BF, isOutput=False)
    cosT = nc.declare_dram_parameter("cosT", [HD, S], BF, isOutput=False)
    sinmT = nc.declare_dram_parameter("sinmT", [HD, S], BF, isOutput=False)
    maskT = nc.declare_dram_parameter("maskT", [128, 128], BF, isOutput=False)
    out = nc.declare_dram_parameter("out", [2 * TCB, D], BF, isOutput=True)

    with tile.TileContext(nc) as tc:
        with tc.tile_pool(name="persist", bufs=1) as persist, \
             tc.tile_pool(name="dram", bufs=1, space="DRAM") as dram, \
             tc.tile_pool(name="psA", bufs=4, space="PSUM") as psA, \
             tc.tile_pool(name="psO", bufs=2, space="PSUM") as psO, \
             tc.tile_pool(name="psD", bufs=2, space="PSUM") as psD, \
             tc.tile_pool(name="expp", bufs=6) as expp, \
             tc.tile_pool(name="spp", bufs=2) as spp, \
             tc.tile_pool(name="yp", bufs=2) as yp, \
             tc.tile_pool(name="recp", bufs=1) as recp, \
             tc.tile_pool(name="xp", bufs=24) as xp, \
             tc.tile_pool(name="wp", bufs=1) as wp, \
             tc.tile_pool(name="tmp", bufs=2) as tmpp, \
             tc.tile_pool(name="wop", bufs=1) as wop, \
             tc.tile_pool(name="ytp", bufs=1) as ytp, \
             tc.tile_pool(name="outp", bufs=2) as outp:

            cos_sb = persist.tile([HD, S], BF, tag="cos")
            sinm_sb = persist.tile([HD, S], BF, tag="sinm")
            mask_sb = persist.tile([128, 128], BF, tag="mask")
            ones_sb = persist.tile([128, 128], BF, tag="ones")
            qT_sb = [persist.tile([128, T], BF, name=f"qT{h}", tag=f"qT{h}") for h in range(HL)]
            kT_sb = [persist.tile([128, T], BF, name=f"kT{h}", tag=f"kT{h}") for h in range(HL)]
            v_sb = [persist.tile([128, HDL], BF, name=f"v{i}", tag=f"v{i}") for i in range(T // 128)]
            # per-(batch, token-half) AllToAll bounce buffers
            a2a_in = [[dram.tile([NCORES, HDL, 128], BF, name=f"a2a_in{b}_{hf}", tag=f"a2a_in{b}_{hf}")
                       for hf in range(2)] for b in range(B)]
            a2a_out = [[dram.tile([NCORES, HDL, 128], BF, name=f"a2a_out{b}_{hf}", tag=f"a2a_out{b}_{hf}")
                        for hf in range(2)] for b in range(B)]

            # PE warmup: the clock needs sustained matmul activity to ramp
            # 1.2->2.4GHz; burn dummy matmuls during the input-DMA lead-in
            # so the real stream starts warm
            warm_sb = persist.tile([128, 256], BF, tag="warm_sb")
            nc.vector.memset(warm_sb[:], 1.0)
            wps = psA.tile([128, 256], F32, tag="a", name="warm_ps")
            for _ in range(10):
                nc.tensor.matmul(wps[:], warm_sb[:, 0:128], warm_sb[:],
                                 start=True, stop=True)

            # warmup collective: absorbs the ~30us first-collective ncfw
            # setup cost while phase 1 computes
            dum_in = dram.tile([NCORES, 1, 128], BF, tag="dum_in")
            dum_out = dram.tile([NCORES, 1, 128], BF, tag="dum_out")
            nc.gpsimd.collective_compute(
                "AllToAll", mybir.AluOpType.bypass,
                ins=[dum_in.opt()], outs=[dum_out.opt()],
                replica_groups=[list(range(NCORES))])

            # ----- input DMAs -----
            # wq/wk lead the two fast HWDGE queues (sync/scalar) so the
            # very first matmul group is not gated behind x tiles; cos/sin
            # lead the gpsimd queue because the first RoPE eviction
            # happens ~5us in.
            w_sb = {}
            for name, wparam in (("q", wq), ("k", wk), ("v", wv)):
                w_sb[name] = wp.tile([128, 16 * HDL], BF, name=f"w{name}", tag=f"w{name}")
            for qd in range(2):
                nc.sync.dma_start(
                    w_sb["q"][:, qd * 2048:(qd + 1) * 2048],
                    wq[:, qd * 2048:(qd + 1) * 2048])
                nc.scalar.dma_start(
                    w_sb["k"][:, qd * 2048:(qd + 1) * 2048],
                    wk[:, qd * 2048:(qd + 1) * 2048])
            nc.gpsimd.dma_start(cos_sb[:], cosT[:, :])
            nc.gpsimd.dma_start(sinm_sb[:], sinmT[:, :])
            for qd in range(2):
                nc.gpsimd.dma_start(
                    w_sb["v"][:, qd * 2048:(qd + 1) * 2048],
                    wv[:, qd * 2048:(qd + 1) * 2048])
            nc.gpsimd.dma_start(mask_sb[:], maskT[:, :])
            nc.vector.memset(ones_sb[:], 1.0)

            # x arrives as [kb, n-chunk] tiles of [128, 512] on the two
            # fast queues (sync/scalar)
            xt = {}
            for th in range(2):
                for n in range(4):
                    xt[th, n] = [xp.tile([128, 512], BF, name=f"xt{th}_{n}_{i}", tag="x")
                                 for i in range(16)]
                    for kb in range(16):
                        eng = (nc.sync, nc.scalar)[kb % 2]
                        eng.dma_start(
                            xt[th, n][kb][:],
                            xT[kb * 128:(kb + 1) * 128,
                               th * 2048 + n * 512: th * 2048 + (n + 1) * 512])

            # wo lands after the x tiles on the scalar queue; first needed
            # ~250us in (output projection)
            wo_sb = wop.tile([128, 16 * D], BF, tag="wo")
            for half in range(2):
                nc.scalar.dma_start(
                    wo_sb[:, half * 8 * D:(half + 1) * 8 * D],
                    wo[:, half * 8 * D:(half + 1) * 8 * D])

            # ----- QKV projection pieces -----
            def qk_proj(th, n):
                # Q, K in d-major layout with fused RoPE on eviction
                for name, dest in (("q", qT_sb), ("k", kT_sb)):
                    for m in range(HL):
                        ps = psA.tile([128, 512], F32, tag="a")
                        for kb in range(16):
                            nc.tensor.matmul(
                                ps[:],
                                w_sb[name][:, kb * HDL + m * 128: kb * HDL + (m + 1) * 128],
                                xt[th, n][kb][:],
                                start=(kb == 0), stop=(kb == 15))
                            if th == 0 and n == 0 and name == "q" and m == 0:
                                # keep the PE busy during the per-tile DMA
                                # waits of the very first chunk so the
                                # clock ramp is not reset
                                nc.tensor.matmul(
                                    wps[:], warm_sb[:, 0:128],
                                    warm_sb[:], start=True, stop=True)
                        t1 = tmpp.tile([128, 512], F32, tag="t1")
                        t2 = tmpp.tile([128, 512], F32, tag="t2")
                        tg = slice(n * 512, (n + 1) * 512)
                        nc.vector.tensor_mul(t1[:], ps[:], cos_sb[:, tg])
                        nc.vector.tensor_mul(t2[0:64, :], ps[64:128, :], sinm_sb[0:64, tg])
                        nc.vector.tensor_mul(t2[64:128, :], ps[0:64, :], sinm_sb[64:128, tg])
                        dst = dest[m][:, th * 2048 + n * 512: th * 2048 + (n + 1) * 512]
                        nc.vector.tensor_add(dst, t1[:], t2[:])

            def v_chunk(th, n, tmi):
                # one token-tile of the V projection (token-major output)
                tm = n * 4 + tmi
                psv = psA.tile([128, 512], F32, tag="a")
                for kb in range(16):
                    nc.tensor.matmul(
                        psv[:, 0:HDL],
                        xt[th, n][kb][:, tmi * 128:(tmi + 1) * 128],
                        w_sb["v"][:, kb * HDL:(kb + 1) * HDL],
                        start=(kb == 0), stop=(kb == 15))
                nc.scalar.copy(v_sb[th * 16 + tm][:], psv[:, 0:HDL])

            # ----- attention pieces -----
            yt_sb = {}

            def attn_qsb(b, qsb):
                """causal attention for one 512-token q chunk, both heads,
                in [k, q] layout with the denominator via a single
                ones-matmul over the DVE-accumulated exp sum."""
                nkb = (qsb + 1) * 4
                for h in range(HL):
                    out_ps = psO.tile([128, 512], F32, tag="o")
                    s_pe = spp.tile([128, 512], BF, tag="s")
                    for kb in range(nkb):
                        j = kb - qsb * 4  # >=0 on the causal diagonal band
                        c0 = max(j, 0) * 128  # first valid q column
                        lg = psA.tile([128, 512], F32, tag="a")
                        nc.tensor.matmul(
                            lg[:, c0:512],
                            kT_sb[h][:, b * S + kb * 128: b * S + (kb + 1) * 128],
                            qT_sb[h][:, b * S + qsb * 512 + c0: b * S + (qsb + 1) * 512],
                            start=True, stop=True)
                        pe = expp.tile([128, 512], BF, tag="e")
                        nc.scalar.activation(pe[:, c0:512], lg[:, c0:512],
                                             AF.Exp, scale=SCALE)
                        if j >= 0:
                            nc.vector.tensor_mul(
                                pe[:, c0:c0 + 128], pe[:, c0:c0 + 128], mask_sb[:])
                        if kb == 0:
                            nc.vector.tensor_copy(s_pe[:], pe[:])
                        else:
                            nc.vector.tensor_add(
                                s_pe[:, c0:512], s_pe[:, c0:512], pe[:, c0:512])
                        nc.tensor.matmul(
                            out_ps[:, c0:512],
                            v_sb[b * 16 + kb][:, h * 128:(h + 1) * 128],
                            pe[:, c0:512],
                            start=(kb == 0), stop=(kb == nkb - 1))
                    den_ps = psD.tile([128, 512], F32, tag="d")
                    nc.tensor.matmul(den_ps[:], ones_sb[:], s_pe[:],
                                     start=True, stop=True)
                    rec = recp.tile([128, 512], F32, tag="r")
                    nc.vector.reciprocal_approx_fast(rec[:], den_ps[:])
                    y = yp.tile([128, 512], BF, tag="y")
                    nc.vector.tensor_mul(y[:], out_ps[:], rec[:])
                    for jj in range(4):
                        nc.sync.dma_start(
                            a2a_in[b][qsb // 2][(qsb % 2) * 4 + jj,
                                                h * 128:(h + 1) * 128, :],
                            y[:, jj * 128:(jj + 1) * 128])

            def emit_a2a(b, hf):
                nc.gpsimd.collective_compute(
                    "AllToAll", mybir.AluOpType.bypass,
                    ins=[a2a_in[b][hf].opt()],
                    outs=[a2a_out[b][hf].opt()],
                    replica_groups=[list(range(NCORES))])
                yt = ytp.tile([128, 16 * 128], BF,
                              name=f"yt{b}_{hf}", tag=f"yt{b}_{hf}")
                nc.sync.dma_start(
                    yt[:],
                    a2a_out[b][hf].rearrange("r (h d) t -> d r h t", h=2))
                yt_sb[b, hf] = yt

            def op_chunk(b, hf, ncol):
                """one 512-model-column slab of the output projection for
                my 128-token chunk of (batch b, half hf)"""
                yt = yt_sb[b, hf]
                ps = psD.tile([128, 512], F32, tag="d")
                for kb in range(16):
                    nc.tensor.matmul(
                        ps[:],
                        yt[:, kb * 128:(kb + 1) * 128],
                        wo_sb[:, kb * D + ncol * 512: kb * D + (ncol + 1) * 512],
                        start=(kb == 0), stop=(kb == 15))
                o = outp.tile([128, 512], BF, tag="ot")
                nc.vector.tensor_copy(o[:], ps[:])
                nc.sync.dma_start(
                    out[b * TCB + hf * 128: b * TCB + hf * 128 + 128,
                        ncol * 512:(ncol + 1) * 512],
                    o[:])

            # ----- emission schedule -----
            # phase 1a: full QKV for batch 0
            for n in range(4):
                qk_proj(0, n)
                for tmi in range(4):
                    v_chunk(0, n, tmi)

            def unit_b1(n):
                # all batch-1 QKV work touching x chunk (1, n), emitted
                # as one unit so x-tile consumption stays monotonic and
                # the x pool can rotate with a small buffer count
                for tmi in range(4):
                    v_chunk(1, n, tmi)
                qk_proj(1, n)

            # phase 2a: batch-0 attention; batch-1 QKV units (independent
            # of batch-0 results) keep the PE fed through the
            # scalar-exp-bound attention stretches
            attn_qsb(0, 0)
            attn_qsb(0, 1)
            emit_a2a(0, 0)
            unit_b1(0)
            attn_qsb(0, 2)
            unit_b1(1)
            attn_qsb(0, 3)
            emit_a2a(0, 1)
            unit_b1(2)
            unit_b1(3)

            # phase 2b: batch-1 attention (hf1 first so its AllToAll
            # issues early), PE gaps filled with output-projection chunks
            # whose AllToAll landed during phase 1b
            attn_qsb(1, 2)
            op_chunk(0, 0, 0)
            op_chunk(0, 0, 1)
            attn_qsb(1, 3)
            emit_a2a(1, 1)
            op_chunk(0, 0, 2)
            op_chunk(0, 0, 3)
            attn_qsb(1, 0)
            op_chunk(0, 1, 0)
            attn_qsb(1, 1)
            emit_a2a(1, 0)
            # tail: remaining output projections; the (1,*) chunks start
            # late enough that their AllToAlls have completed
            for ncol in range(1, 4):
                op_chunk(0, 1, ncol)
            for ncol in range(4):
                op_chunk(1, 1, ncol)
            for ncol in range(4):
                op_chunk(1, 0, ncol)
    nc.finalize()
    return nc


_CACHE = {}


def _get_nc():
    if "nc" not in _CACHE:
        _CACHE["nc"] = build()
    return _CACHE["nc"]


def _pretile(w, cols):
    """[2048, cols] -> [128, 16*cols], kb-major along columns."""
    return np.ascontiguousarray(
        w.reshape(16, 128, cols).transpose(1, 0, 2).reshape(128, 16 * cols)
    ).astype(BF16)


def _prep_in_maps(x, freq_cos, freq_sin, wq, wk, wv, wo):
    xTa = np.asarray(x, np.float32).reshape(T, D).T.astype(BF16)
    cos_t = np.asarray(freq_cos, np.float32).T  # [64, S]
    sin_t = np.asarray(freq_sin, np.float32).T
    cosT = np.concatenate([cos_t, cos_t], 0).astype(BF16)
    sinmT = np.concatenate([-sin_t, sin_t], 0).astype(BF16)
    kk = np.arange(128)[:, None]
    qq = np.arange(128)[None, :]
    maskTa = (qq >= kk).astype(BF16)
    wo_t = _pretile(np.asarray(wo, np.float32), D)
    wq = np.asarray(wq, np.float32)
    wk = np.asarray(wk, np.float32)
    wv = np.asarray(wv, np.float32)
    in_maps = []
    for c in range(NCORES):
        sl = slice(c * HDL, (c + 1) * HDL)
        in_maps.append({
            "xT": xTa,
            "wq": _pretile(wq[:, sl], HDL),
            "wk": _pretile(wk[:, sl], HDL),
            "wv": _pretile(wv[:, sl], HDL),
            "wo": wo_t,
            "cosT": cosT,
            "sinmT": sinmT,
            "maskT": maskTa,
        })
    return in_maps


def kernel(x, freq_cos, freq_sin, wq, wk, wv, wo, _trace=False):
    nc = _get_nc()
    in_maps = _prep_in_maps(x, freq_cos, freq_sin, wq, wk, wv, wo)
    try:
        res = run_bass_kernel_spmd(
            nc, in_maps, core_ids=list(range(NCORES)), trace=_trace)
    except Exception:
        # transient NRT_EXEC_UNIT_UNRECOVERABLE has been observed to clear
        # on a retry
        res = run_bass_kernel_spmd(
            nc, in_maps, core_ids=list(range(NCORES)), trace=_trace)
    # core c holds tokens [hf*1024 + c*128) of each (batch, half)
    full = np.empty((B, S, D), np.float32)
    for c in range(NCORES):
        o = np.asarray(res.results[c]["out"], np.float32)
        for b in range(B):
            for hf in range(2):
                full[b, hf * 1024 + c * 128: hf * 1024 + (c + 1) * 128] = \
                    o[b * TCB + hf * 128: b * TCB + hf * 128 + 128]
    if _trace:
        return full, res
    return full
